# revision 2
# baseline (speedup 1.0000x reference)
"""Trainium2 Bass kernel for nn_AV_Attention (dense transformer block).

Computes, for each batch b (data-parallel, one batch per NeuronCore):
    Q = img @ q_w.T + q_b          [S, K]
    K = text @ k_w.T + k_b         [S, K]
    V = text @ v_w.T + v_b         [S, V]
    scores = Q @ K.T               [S, S]
    atten  = softmax(scores) / sqrt(K)
    output = atten @ V             [S, V]
    feature = output + text
Returns (output, feature), both [B, S, V].

Per-core plan (all matmuls fp32r = full-rate PE):
  phase 1a: Q^T = qwT-stationary matmuls over img^T  -> HBM scratch
  phase 1b: K^T (SBUF-resident [k,s]) and V (SBUF-resident [s,v]) from text^T
  phase 2:  per 128-row q-tile: scores = Q^T.T@K^T -> exp (no max-sub; scores
            bounded ~|60| so exp stays finite in fp32) with fused row-sum ->
            PE-transpose P -> P^T-stationary AV matmuls -> scale by
            (1/32)/rowsum -> +text residual.
"""
import sys
import os

for _p in ("/opt/trn_rl_repo", "/root/.axon_site/_ro/trn_rl_repo"):
    if os.path.isdir(_p) and _p not in sys.path:
        sys.path.insert(0, _p)

import numpy as np

import concourse.bass as bass
import concourse.tile as tile
import concourse.mybir as mybir
from concourse import bacc
from concourse.bass import ds
from concourse.bass_utils import run_bass_kernel_spmd
from concourse.masks import make_identity

B, S, D = 8, 2048, 1024
KD = 1024   # dim_k
VD = 1024   # dim_v
P = 128
NORM = 1.0 / float(np.sqrt(KD))

f32 = mybir.dt.float32
f32r = mybir.dt.float32r

Copy = mybir.ActivationFunctionType.Copy
Ident = mybir.ActivationFunctionType.Identity
Exp = mybir.ActivationFunctionType.Exp
X = mybir.AxisListType.X
ADD = mybir.AluOpType.add
MULT = mybir.AluOpType.mult


def build_nc(nrep: int = 1):
    nc = bacc.Bacc(None, target_bir_lowering=False, debug=False)

    imgT = nc.dram_tensor("imgT", (D, S), f32r, kind="ExternalInput")
    textT = nc.dram_tensor("textT", (D, S), f32r, kind="ExternalInput")
    textn = nc.dram_tensor("textn", (S, D), f32, kind="ExternalInput")
    qwT = nc.dram_tensor("qwT", (D, KD), f32r, kind="ExternalInput")
    kwT = nc.dram_tensor("kwT", (D, KD), f32r, kind="ExternalInput")
    vwT = nc.dram_tensor("vwT", (D, VD), f32r, kind="ExternalInput")
    qb = nc.dram_tensor("qb", (KD,), f32, kind="ExternalInput")
    kb = nc.dram_tensor("kb", (KD,), f32, kind="ExternalInput")
    vb = nc.dram_tensor("vb", (VD,), f32, kind="ExternalInput")
    out = nc.dram_tensor("out", (S, VD), f32, kind="ExternalOutput")
    feat = nc.dram_tensor("feat", (S, VD), f32, kind="ExternalOutput")

    with tile.TileContext(nc) as tc:
        with tc.tile_pool(name="const", bufs=1) as const, \
             tc.tile_pool(name="dram", bufs=1, space="DRAM") as dram:
            ident = const.tile([P, P], f32)
            make_identity(nc, ident)
            qb_sb = const.tile([P, 8], f32)
            kb_sb = const.tile([P, 8], f32)
            nc.sync.dma_start(qb_sb, qb[:].rearrange("(t p) -> p t", p=P))
            nc.sync.dma_start(kb_sb, kb[:].rearrange("(t p) -> p t", p=P))
            vb_bc = const.tile([P, VD], f32)
            nc.sync.dma_start(
                vb_bc, bass.AP(vb, 0, [[0, P], [1, VD]]))

            qt_hbm = dram.tile([KD, S], f32r)
            qt_view = qt_hbm.rearrange("(t p) s -> p t s", p=P)

            for _rep in range(nrep):
                # ---------------- phase 1a: Q^T -> HBM ----------------
                with tc.tile_pool(name="ph1a", bufs=1) as ph1a, \
                     tc.tile_pool(name="st1a", bufs=3) as st1a, \
                     tc.tile_pool(name="ps1a", bufs=2, space="PSUM") as ps1a:
                    imgT_sb = ph1a.tile([P, 8, S], f32r)
                    qw_sb = ph1a.tile([P, 8, KD], f32r)
                    for dt in range(8):
                        nc.sync.dma_start(imgT_sb[:, dt, :], imgT[ds(dt * P, P), :])
                        nc.sync.dma_start(qw_sb[:, dt, :], qwT[ds(dt * P, P), :])
                    for kt in range(8):
                        ps = ps1a.tile([P, S], f32, tag="ps")
                        for dt in range(8):
                            for c in range(4):
                                nc.tensor.matmul(
                                    ps[:, ds(c * 512, 512)],
                                    qw_sb[:, dt, ds(kt * P, P)],
                                    imgT_sb[:, dt, ds(c * 512, 512)],
                                    start=(dt == 0), stop=(dt == 7))
                        stg = st1a.tile([P, S], f32r, tag="stg")
                        nc.scalar.activation(stg, ps, Ident, bias=qb_sb[:, kt:kt + 1])
                        nc.sync.dma_start(qt_hbm[ds(kt * P, P), :], stg)

                # ---------------- phase 1b: K^T + V resident ----------------
                with tc.tile_pool(name="persist", bufs=1) as persist:
                    kT_sb = persist.tile([P, 8, S], f32r)
                    v_sb = persist.tile([P, 16, VD], f32r)
                    with tc.tile_pool(name="ph1b", bufs=1) as ph1b, \
                         tc.tile_pool(name="w1b", bufs=1) as w1b, \
                         tc.tile_pool(name="ps1b", bufs=2, space="PSUM") as ps1b:
                        H = S // 2
                        for h in range(2):
                            textT_h = ph1b.tile([P, 8, H], f32r, tag="textT")
                            for dt in range(8):
                                nc.sync.dma_start(
                                    textT_h[:, dt, :],
                                    textT[ds(dt * P, P), ds(h * H, H)])
                            kw_sb = w1b.tile([P, 8, KD], f32r, tag="w")
                            for dt in range(8):
                                nc.sync.dma_start(kw_sb[:, dt, :], kwT[ds(dt * P, P), :])
                            for kt in range(8):
                                ps = ps1b.tile([P, H], f32, tag="psk")
                                for dt in range(8):
                                    for c in range(2):
                                        nc.tensor.matmul(
                                            ps[:, ds(c * 512, 512)],
                                            kw_sb[:, dt, ds(kt * P, P)],
                                            textT_h[:, dt, ds(c * 512, 512)],
                                            start=(dt == 0), stop=(dt == 7))
                                nc.scalar.activation(
                                    kT_sb[:, kt, ds(h * H, H)], ps, Ident,
                                    bias=kb_sb[:, kt:kt + 1])
                            vw_sb = w1b.tile([P, 8, VD], f32r, tag="w")
                            for dt in range(8):
                                nc.sync.dma_start(vw_sb[:, dt, :], vwT[ds(dt * P, P), :])
                            for sti in range(8):
                                st = h * 8 + sti
                                psv = ps1b.tile([P, VD], f32, tag="psv")
                                for dt in range(8):
                                    for c in range(2):
                                        nc.tensor.matmul(
                                            psv[:, ds(c * 512, 512)],
                                            textT_h[:, dt, ds(sti * P, P)],
                                            vw_sb[:, dt, ds(c * 512, 512)],
                                            start=(dt == 0), stop=(dt == 7))
                                nc.vector.scalar_tensor_tensor(
                                    v_sb[:, st, :], psv, 1.0, vb_bc,
                                    op0=MULT, op1=ADD)

                    # ---------------- phase 2: attention ----------------
                    with tc.tile_pool(name="ph2q", bufs=3) as ph2q, \
                         tc.tile_pool(name="ph2p", bufs=2) as ph2p, \
                         tc.tile_pool(name="ph2o", bufs=2) as ph2o, \
                         tc.tile_pool(name="small", bufs=4) as small, \
                         tc.tile_pool(name="ps_s", bufs=2, space="PSUM") as ps_s, \
                         tc.tile_pool(name="ps_av", bufs=1, space="PSUM") as ps_av, \
                         tc.tile_pool(name="ps_tr", bufs=3, space="PSUM") as ps_tr:
                        for qt in range(16):
                            qT_t = ph2q.tile([P, 8, P], f32r, tag="qT")
                            nc.sync.dma_start(qT_t, qt_view[:, :, ds(qt * P, P)])
                            textn_t = ph2q.tile([P, VD], f32, tag="tx")
                            nc.sync.dma_start(textn_t, textn[ds(qt * P, P), :])

                            p_sb = ph2p.tile([P, S], f32r, tag="p")
                            sums = small.tile([P, 4], f32, tag="sums")
                            for sc in range(4):
                                sps = ps_s.tile([P, 512], f32, tag="s")
                                for kt in range(8):
                                    nc.tensor.matmul(
                                        sps, qT_t[:, kt, :],
                                        kT_sb[:, kt, ds(sc * 512, 512)],
                                        start=(kt == 0), stop=(kt == 7))
                                nc.scalar.activation(
                                    p_sb[:, ds(sc * 512, 512)], sps, Exp,
                                    accum_out=sums[:, sc:sc + 1])

                            tot = small.tile([P, 1], f32, tag="tot")
                            nc.vector.tensor_reduce(tot, sums, axis=X, op=ADD)
                            scl = small.tile([P, 1], f32, tag="scl")
                            nc.vector.reciprocal(scl, tot)
                            scl2 = small.tile([P, 1], f32, tag="scl2")
                            nc.vector.tensor_scalar_mul(scl2, scl, NORM)

                            pT = ph2p.tile([P, 16, P], f32r, tag="pT")
                            for g in range(4):
                                trp = ps_tr.tile([P, 4, P], f32, tag="tr")
                                for j in range(4):
                                    st = g * 4 + j
                                    nc.tensor.transpose(
                                        trp[:, j, :],
                                        p_sb[:, ds(st * P, P)].bitcast(f32),
                                        ident)
                                nc.scalar.copy(pT[:, ds(g * 4, 4), :], trp)

                            avp = ps_av.tile([P, VD], f32, tag="av")
                            for st in range(16):
                                for c in range(2):
                                    nc.tensor.matmul(
                                        avp[:, ds(c * 512, 512)],
                                        pT[:, st, :],
                                        v_sb[:, st, ds(c * 512, 512)],
                                        start=(st == 0), stop=(st == 15))

                            out_t = ph2o.tile([P, VD], f32, tag="out")
                            feat_t = ph2o.tile([P, VD], f32, tag="feat")
                            nc.vector.tensor_scalar_mul(out_t, avp, scl2)
                            nc.vector.tensor_add(feat_t, out_t, textn_t)
                            nc.sync.dma_start(out[ds(qt * P, P), :], out_t)
                            nc.sync.dma_start(feat[ds(qt * P, P), :], feat_t)

    nc.finalize()
    return nc


_NC_CACHE = {}


def _get_nc(nrep: int = 1):
    if nrep not in _NC_CACHE:
        _NC_CACHE[nrep] = build_nc(nrep)
    return _NC_CACHE[nrep]


def make_in_maps(img, text, q_w, q_b, k_w, k_b, v_w, v_b):
    img = np.ascontiguousarray(np.asarray(img, dtype=np.float32))
    text = np.ascontiguousarray(np.asarray(text, dtype=np.float32))
    qwT = np.ascontiguousarray(np.asarray(q_w, np.float32).T)
    kwT = np.ascontiguousarray(np.asarray(k_w, np.float32).T)
    vwT = np.ascontiguousarray(np.asarray(v_w, np.float32).T)
    q_b = np.ascontiguousarray(np.asarray(q_b, np.float32))
    k_b = np.ascontiguousarray(np.asarray(k_b, np.float32))
    v_b = np.ascontiguousarray(np.asarray(v_b, np.float32))
    in_maps = []
    for b in range(B):
        in_maps.append({
            "imgT": np.ascontiguousarray(img[b].T),
            "textT": np.ascontiguousarray(text[b].T),
            "textn": text[b],
            "qwT": qwT, "kwT": kwT, "vwT": vwT,
            "qb": q_b, "kb": k_b, "vb": v_b,
        })
    return in_maps


def kernel(img, text, q_w, q_b, k_w, k_b, v_w, v_b):
    in_maps = make_in_maps(img, text, q_w, q_b, k_w, k_b, v_w, v_b)
    nc = _get_nc(1)
    res = run_bass_kernel_spmd(nc, in_maps, core_ids=list(range(B)))
    output = np.stack([r["out"] for r in res.results]).astype(np.float32)
    feature = np.stack([r["feat"] for r in res.results]).astype(np.float32)
    return output, feature


if __name__ == "__main__":
    # quick self-check with random data
    rng = np.random.default_rng(0)
    ins = {
        "img": rng.standard_normal((B, S, D), dtype=np.float32),
        "text": rng.standard_normal((B, S, D), dtype=np.float32),
        "q_w": (rng.random((KD, D), dtype=np.float32) - 0.5) / 16,
        "q_b": (rng.random(KD, dtype=np.float32) - 0.5) / 16,
        "k_w": (rng.random((KD, D), dtype=np.float32) - 0.5) / 16,
        "k_b": (rng.random(KD, dtype=np.float32) - 0.5) / 16,
        "v_w": (rng.random((VD, D), dtype=np.float32) - 0.5) / 16,
        "v_b": (rng.random(VD, dtype=np.float32) - 0.5) / 16,
    }
    o, f = kernel(**ins)
    print("out", o.shape, o.dtype, "feat", f.shape)


# revision 23
# speedup vs baseline: 559.4058x; 559.4058x over previous
"""Trainium2 Bass kernel for nn_AV_Attention (dense transformer block).

Computes, for each batch b (data-parallel, one batch per NeuronCore):
    Q = img @ q_w.T + q_b          [S, K]
    K = text @ k_w.T + k_b         [S, K]
    V = text @ v_w.T + v_b         [S, V]
    scores = Q @ K.T               [S, S]
    atten  = softmax(scores) / sqrt(K)
    output = atten @ V             [S, V]
    feature = output + text
Returns (output, feature), both [B, S, V].

Per-core plan (all matmuls fp32r = full-rate PE):
  phase 1a: Q^T = qwT-stationary matmuls over img^T  -> HBM scratch
  phase 1b: K^T (SBUF-resident [k,s]) and V (SBUF-resident [s,v]) from text^T
  phase 2:  per 128-row q-tile: scores = Q^T.T@K^T -> exp (no max-sub; scores
            bounded ~|60| so exp stays finite in fp32) with fused row-sum ->
            PE-transpose P -> P^T-stationary AV matmuls -> scale by
            (1/32)/rowsum -> +text residual.
"""
import sys
import os
import time

for _p in ("/opt/trn_rl_repo", "/root/.axon_site/_ro/trn_rl_repo"):
    if os.path.isdir(_p) and _p not in sys.path:
        sys.path.insert(0, _p)

import numpy as np

import concourse.bass as bass
import concourse.tile as tile
import concourse.mybir as mybir
from concourse import bacc
from concourse.bass import ds
from concourse.bass_utils import run_bass_kernel_spmd
from concourse.masks import make_identity

B, S, D = 8, 2048, 1024
KD = 1024   # dim_k
VD = 1024   # dim_v
P = 128
NORM = 1.0 / float(np.sqrt(KD))

f32 = mybir.dt.float32
f32r = mybir.dt.float32r

Copy = mybir.ActivationFunctionType.Copy
Ident = mybir.ActivationFunctionType.Identity
Exp = mybir.ActivationFunctionType.Exp
X = mybir.AxisListType.X
ADD = mybir.AluOpType.add
MULT = mybir.AluOpType.mult


def build_nc(nrep: int = 1):
    nc = bacc.Bacc(None, target_bir_lowering=False, debug=False)

    imgT = nc.dram_tensor("imgT", (D, S), f32r, kind="ExternalInput")
    textT = nc.dram_tensor("textT", (D, S), f32r, kind="ExternalInput")
    textn = nc.dram_tensor("textn", (S, D), f32, kind="ExternalInput")
    qwR = nc.dram_tensor("qwR", (8, P, 8, P), f32r, kind="ExternalInput")
    kwR = nc.dram_tensor("kwR", (8, P, 8, P), f32r, kind="ExternalInput")
    vwT = nc.dram_tensor("vwT", (D, VD), f32r, kind="ExternalInput")
    qb = nc.dram_tensor("qb", (KD,), f32, kind="ExternalInput")
    kb = nc.dram_tensor("kb", (KD,), f32, kind="ExternalInput")
    vb = nc.dram_tensor("vb", (VD,), f32, kind="ExternalInput")
    out = nc.dram_tensor("out", (S, VD), f32, kind="ExternalOutput")
    feat = nc.dram_tensor("feat", (S, VD), f32, kind="ExternalOutput")

    with tile.TileContext(nc) as tc:
        with tc.tile_pool(name="const", bufs=1) as const, \
             tc.tile_pool(name="dram", bufs=1, space="DRAM") as dram:
            ident = const.tile([P, P], f32)
            make_identity(nc, ident)
            ident_r = const.tile([P, P], f32r)
            nc.scalar.copy(ident_r, ident)
            qb_sb = const.tile([P, 8], f32)
            kb_sb = const.tile([P, 8], f32)
            nc.gpsimd.dma_start(qb_sb, qb[:].rearrange("(t p) -> p t", p=P))
            nc.gpsimd.dma_start(kb_sb, kb[:].rearrange("(t p) -> p t", p=P))
            qt_hbm = dram.tile([KD, S], f32r)
            qt_view = qt_hbm.rearrange("(t p) s -> p t s", p=P)

            for _rep in range(nrep):
                # ---------------- phase 1a: Q^T -> HBM ----------------
                # img^T streamed in S-quarters (prefetched); qw resident.
                with tc.tile_pool(name="ph1a", bufs=1) as ph1a, \
                     tc.tile_pool(name="img1a", bufs=2) as img1a, \
                     tc.tile_pool(name="st1a", bufs=3) as st1a, \
                     tc.tile_pool(name="ps1a", bufs=2, space="PSUM") as ps1a:
                    imgT_v = imgT.rearrange("(t p) s -> p t s", p=P)
                    imgq = [None, None]
                    imgq[0] = img1a.tile([P, 8, 512], f32r, tag="img", name="img0")
                    qw_sb = ph1a.tile([P, 8, 8, P], f32r)
                    nc.sync.dma_start(qw_sb[:, 0], qwR[0])
                    for dt in range(8):
                        nc.sync.dma_start(imgq[0][:, dt, :],
                                          imgT_v[:, dt, ds(0, 512)])
                    for kt in range(1, 8):
                        nc.sync.dma_start(qw_sb[:, kt], qwR[kt])
                    for q in range(4):
                        imgT_q = imgq[q % 2]
                        if q < 3:
                            nxt = img1a.tile([P, 8, 512], f32r, tag="img",
                                             name=f"img{q + 1}")
                            imgq[(q + 1) % 2] = nxt
                            for dt in range(8):
                                nc.sync.dma_start(
                                    nxt[:, dt, :],
                                    imgT_v[:, dt, ds((q + 1) * 512, 512)])
                        for kt in range(8):
                            ps = ps1a.tile([P, 512], f32, tag="ps")
                            for dt in range(8):
                                nc.tensor.matmul(
                                    ps, qw_sb[:, kt, dt, :],
                                    imgT_q[:, dt, :],
                                    start=(dt == 0), stop=(dt == 7))
                            stg = st1a.tile([P, 512], f32r, tag="stg")
                            nc.scalar.activation(stg, ps, Ident,
                                                 bias=qb_sb[:, kt:kt + 1])
                            nc.sync.dma_start(
                                qt_hbm[ds(kt * P, P), ds(q * 512, 512)], stg)

                # ---------------- phase 1b: K^T + V resident ----------------
                # text^T streamed in S-quarters; vw resident (loaded once);
                # kw streamed as per-kt [d, 128] slices.
                with tc.tile_pool(name="persist", bufs=1) as persist:
                    kT_sb = persist.tile([P, 8, S], f32r)
                    v_sb = persist.tile([P, 16, VD], f32r)
                    with tc.tile_pool(name="ph1b", bufs=1) as ph1b, \
                         tc.tile_pool(name="tx1b", bufs=2) as tx1b, \
                         tc.tile_pool(name="kw1b", bufs=2) as kw1b, \
                         tc.tile_pool(name="ps1b", bufs=2, space="PSUM") as ps1b:
                        vw_sb = ph1b.tile([P, 8, VD], f32r)
                        vb_bc = ph1b.tile([P, VD], f32)
                        textT_v = textT.rearrange("(t p) s -> p t s", p=P)
                        txq = [tx1b.tile([P, 8, 512], f32r, tag="textT",
                                         name=f"tx{q}") for q in range(2)]
                        nc.sync.dma_start(txq[0], textT_v[:, :, ds(0, 512)])
                        vwT_v = vwT.rearrange("(t p) v -> p t v", p=P)
                        for q in range(4):
                            textT_q = txq[q % 2]
                            if q < 3:
                                nxt = tx1b.tile([P, 8, 512], f32r, tag="textT",
                                                name=f"tx{q + 1}")
                                txq[(q + 1) % 2] = nxt
                                nc.sync.dma_start(
                                    nxt, textT_v[:, :, ds((q + 1) * 512, 512)])
                            if q == 0:
                                nc.sync.dma_start(
                                    vb_bc, bass.AP(vb, 0, [[0, P], [1, VD]]))
                                for vh in range(4):
                                    nc.sync.dma_start(
                                        vw_sb[:, ds(vh * 2, 2), :],
                                        vwT_v[:, ds(vh * 2, 2), :])
                            for kt in range(8):
                                kw_sl = kw1b.tile([P, 8, P], f32r, tag="kw")
                                nc.sync.dma_start(kw_sl, kwR[kt])
                                ps = ps1b.tile([P, 512], f32, tag="psk")
                                for dt in range(8):
                                    nc.tensor.matmul(
                                        ps, kw_sl[:, dt, :],
                                        textT_q[:, dt, :],
                                        start=(dt == 0), stop=(dt == 7))
                                nc.scalar.activation(
                                    kT_sb[:, kt, ds(q * 512, 512)], ps, Ident,
                                    bias=kb_sb[:, kt:kt + 1])
                            for sti in range(4):
                                st = q * 4 + sti
                                psv = ps1b.tile([P, VD], f32, tag="psv")
                                for dt in range(8):
                                    for c in range(2):
                                        nc.tensor.matmul(
                                            psv[:, ds(c * 512, 512)],
                                            textT_q[:, dt, ds(sti * P, P)],
                                            vw_sb[:, dt, ds(c * 512, 512)],
                                            start=(dt == 0), stop=(dt == 7))
                                nc.vector.scalar_tensor_tensor(
                                    v_sb[:, st, :], psv, 1.0, vb_bc,
                                    op0=MULT, op1=ADD)

                    # ---------------- phase 2: attention ----------------
                    with tc.tile_pool(name="ph2q", bufs=3) as ph2q, \
                         tc.tile_pool(name="ph2p", bufs=2) as ph2p, \
                         tc.tile_pool(name="ph2o", bufs=2) as ph2o, \
                         tc.tile_pool(name="small", bufs=4) as small, \
                         tc.tile_pool(name="ps_s", bufs=2, space="PSUM") as ps_s, \
                         tc.tile_pool(name="ps_av", bufs=1, space="PSUM") as ps_av, \
                         tc.tile_pool(name="ps_tr", bufs=3, space="PSUM") as ps_tr:
                        for qt in range(16):
                            if qt % 2 == 0:
                                qT_pair = ph2q.tile([P, 8, 256], f32r, tag="qT", bufs=2)
                                for dt in range(8):
                                    nc.sync.dma_start(
                                        qT_pair[:, dt, :],
                                        qt_view[:, dt, ds(qt * P, 256)])
                            qT_t = qT_pair[:, :, ds((qt % 2) * P, P)]
                            textn_t = ph2q.tile([P, VD], f32, tag="tx", bufs=1)
                            nc.sync.dma_start(textn_t, textn[ds(qt * P, P), :])

                            p_sb = ph2p.tile([P, S], f32r, tag="p")
                            sums = small.tile([P, 4], f32, tag="sums")
                            for sc in range(4):
                                sps = ps_s.tile([P, 512], f32, tag="s")
                                for kt in range(8):
                                    nc.tensor.matmul(
                                        sps, qT_t[:, kt, :],
                                        kT_sb[:, kt, ds(sc * 512, 512)],
                                        start=(kt == 0), stop=(kt == 7))
                                nc.scalar.activation(
                                    p_sb[:, ds(sc * 512, 512)], sps, Exp,
                                    accum_out=sums[:, sc:sc + 1])

                            tot = small.tile([P, 1], f32, tag="tot")
                            nc.vector.tensor_reduce(tot, sums, axis=X, op=ADD)
                            scl = small.tile([P, 1], f32, tag="scl")
                            nc.vector.reciprocal(scl, tot)
                            scl2 = small.tile([P, 1], f32, tag="scl2")
                            nc.vector.tensor_scalar_mul(scl2, scl, NORM)

                            pT = ph2p.tile([P, 16, P], f32r, tag="pT")
                            for g in range(4):
                                trp = ps_tr.tile([P, 4, P], f32r, tag="tr")
                                for j in range(4):
                                    st = g * 4 + j
                                    nc.tensor.transpose(
                                        trp[:, j, :],
                                        p_sb[:, ds(st * P, P)],
                                        ident_r)
                                nc.scalar.copy(pT[:, ds(g * 4, 4), :], trp)

                            avp = ps_av.tile([P, VD], f32, tag="av")
                            for st in range(16):
                                for c in range(2):
                                    nc.tensor.matmul(
                                        avp[:, ds(c * 512, 512)],
                                        pT[:, st, :],
                                        v_sb[:, st, ds(c * 512, 512)],
                                        start=(st == 0), stop=(st == 15))

                            out_t = ph2o.tile([P, VD], f32, tag="out")
                            feat_t = ph2o.tile([P, VD], f32, tag="feat")
                            nc.vector.tensor_scalar_mul(out_t, avp, scl2)
                            nc.vector.tensor_add(feat_t, out_t, textn_t)
                            nc.sync.dma_start(out[ds(qt * P, P), :], out_t)
                            nc.sync.dma_start(feat[ds(qt * P, P), :], feat_t)

    nc.finalize()
    return nc


_NC_CACHE = {}


def _get_nc(nrep: int = 1):
    if nrep not in _NC_CACHE:
        _NC_CACHE[nrep] = build_nc(nrep)
    return _NC_CACHE[nrep]


def make_in_maps(img, text, q_w, q_b, k_w, k_b, v_w, v_b):
    img = np.ascontiguousarray(np.asarray(img, dtype=np.float32))
    text = np.ascontiguousarray(np.asarray(text, dtype=np.float32))
    qwT = np.asarray(q_w, np.float32).T
    qwR = np.ascontiguousarray(
        qwT.reshape(8, P, 8, P).transpose(2, 1, 0, 3))
    kwT = np.asarray(k_w, np.float32).T
    kwR = np.ascontiguousarray(
        kwT.reshape(8, P, 8, P).transpose(2, 1, 0, 3))
    vwT = np.ascontiguousarray(np.asarray(v_w, np.float32).T)
    q_b = np.ascontiguousarray(np.asarray(q_b, np.float32))
    k_b = np.ascontiguousarray(np.asarray(k_b, np.float32))
    v_b = np.ascontiguousarray(np.asarray(v_b, np.float32))
    in_maps = []
    for b in range(B):
        in_maps.append({
            "imgT": np.ascontiguousarray(img[b].T),
            "textT": np.ascontiguousarray(text[b].T),
            "textn": text[b],
            "qwR": qwR, "kwR": kwR, "vwT": vwT,
            "qb": q_b, "kb": k_b, "vb": v_b,
        })
    return in_maps


def kernel(img, text, q_w, q_b, k_w, k_b, v_w, v_b):
    in_maps = make_in_maps(img, text, q_w, q_b, k_w, k_b, v_w, v_b)
    nc = _get_nc(1)
    try:
        res = run_bass_kernel_spmd(nc, in_maps, core_ids=list(range(B)))
    except Exception:
        # transient device wedge (NRT timeout etc): one retry
        time.sleep(10)
        res = run_bass_kernel_spmd(nc, in_maps, core_ids=list(range(B)))
    output = np.stack([r["out"] for r in res.results]).astype(np.float32)
    feature = np.stack([r["feat"] for r in res.results]).astype(np.float32)
    return output, feature


if __name__ == "__main__":
    # quick self-check with random data
    rng = np.random.default_rng(0)
    ins = {
        "img": rng.standard_normal((B, S, D), dtype=np.float32),
        "text": rng.standard_normal((B, S, D), dtype=np.float32),
        "q_w": (rng.random((KD, D), dtype=np.float32) - 0.5) / 16,
        "q_b": (rng.random(KD, dtype=np.float32) - 0.5) / 16,
        "k_w": (rng.random((KD, D), dtype=np.float32) - 0.5) / 16,
        "k_b": (rng.random(KD, dtype=np.float32) - 0.5) / 16,
        "v_w": (rng.random((VD, D), dtype=np.float32) - 0.5) / 16,
        "v_b": (rng.random(VD, dtype=np.float32) - 0.5) / 16,
    }
    o, f = kernel(**ins)
    print("out", o.shape, o.dtype, "feat", f.shape)
